# revision 12
# baseline (speedup 1.0000x reference)
"""DeformUnfold (3x3, pad 1, stride 1, dil 1, DG 1) on TRN2, batch-parallel
over 8 NeuronCores.

Input  x      [8, 64, 128, 128] f32
       offset [8, 18, 128, 128] f32
Output        [8, 576, 16384]   f32  (C*K x Ho*Wo unfold, channel-major)

Per core (= one batch element), sample-major SWDGE gather design:
 - Host packs xpack[s, tb*128 + c*2 + lr] = x[c, min(y+tb,127), min(x+lr,127)]
   (s = y*128+x): the full 2x2 bilinear footprint of spatial position s as
   one contiguous 512B fp16 block in HBM.
 - Host folds offsets into per-sample int16 gather indices (corner block
   row, border-clamped with weight-swap trick) and 4 fp16 corner weights.
 - Device, per quarter-tap chunk of J=4096 samples: 4 dma_gathers of 1024
   indices (SWDGE descriptor-carveout limit) land sample j at SBUF
   partition j%128, slot j//128 -> G[128, 32, 256].  SWDGE gather is
   descriptor-bound (~8ns/desc per queue, ~3.6ns/desc with 4 queues), so
   chunks round-robin over all 4 SWDGE queues with a 4-deep G pool to keep
   every queue busy.  DVE: one in-place tensor_mul against the weight tile
   broadcast across the 64-channel dim via a stride-0 middle AP dim (keeps
   the packed-fp16 2x mode, which only constrains the innermost dim), one
   packed fp16 add folding top+bottom rows, one strided add folding
   left+right that emits f32 directly.
 - Output DMA writes [p][c][g] -> out[c*9+k, q*4096 + p*32 + g].
No SBUF staging of x; gathers read HBM directly.  Per-core: gather 75.5MB
(147456 descriptors), DVE ~295K cycles, out 37.75MB, all overlapped;
descriptor generation on the Pool engine is the bottleneck.
"""

import contextlib

import numpy as np
import ml_dtypes  # noqa: F401

import concourse.bacc as bacc
import concourse.mybir as mybir
import concourse.tile as tile
from concourse.bass_utils import run_bass_kernel_spmd

B, C, H, W = 8, 64, 128, 128
K = 9
HW = H * W
J = 4096            # gather indices per chunk (quarter tap)
SUBJ = 1024         # indices per dma_gather call (descriptor-carveout limit)
GC = J // 128       # 32 free slots per partition
NQ = 4              # quarter-taps per tap == SWDGE queues
NCHUNK = K * NQ
DT = mybir.dt

_cache = {}


def _build_nc(repeat=1):
    ckey = ("nc", repeat)
    if ckey in _cache:
        return _cache[ckey]
    nc = bacc.Bacc(
        "TRN2", target_bir_lowering=False, debug=False, num_swdge_queues=NQ
    )
    xp_ext = nc.declare_dram_parameter("xp", [HW, 256], DT.float16, isOutput=False)
    idx_ext = nc.declare_dram_parameter(
        "idx", [128, NCHUNK * (J // 16)], DT.int16, isOutput=False
    )
    w_ext = nc.declare_dram_parameter(
        "w", [128, NCHUNK * GC * 4], DT.float16, isOutput=False
    )
    out_ext = nc.declare_dram_parameter("out", [C * K, HW], DT.float32, isOutput=True)
    # out[c*K + k, q*4096 + p*32 + g]
    out_v = out_ext[:].rearrange("(c k) (q p g) -> k q p c g", k=K, q=NQ, p=128)

    with tile.TileContext(nc) as tc:
        with (
            tc.tile_pool(name="gp", bufs=NQ) as gp,
            tc.tile_pool(name="ip", bufs=4) as ip,
            tc.tile_pool(name="wp", bufs=4) as wp,
            tc.tile_pool(name="up", bufs=4) as up,
            tc.tile_pool(name="vp", bufs=4) as vp,
        ):
            loop_cm = (
                tc.For_i(0, repeat, 1) if repeat > 1 else contextlib.nullcontext()
            )
            with loop_cm:
                for t in range(K):
                    for q in range(NQ):
                        blk = t * NQ + q
                        itt = ip.tile([128, J // 16], DT.int16, tag="idx")
                        nc.sync.dma_start(
                            out=itt[:],
                            in_=idx_ext[:, blk * (J // 16) : (blk + 1) * (J // 16)],
                        )
                        it = itt[:]
                        wtt = wp.tile([128, GC * 4], DT.float16, tag="w")
                        nc.sync.dma_start(
                            out=wtt[:],
                            in_=w_ext[:, blk * (GC * 4) : (blk + 1) * (GC * 4)],
                        )
                        wt = wtt[:]
                        G = gp.tile([128, GC, 256], DT.float16, tag="g")
                        for s in range(J // SUBJ):
                            nc.gpsimd.dma_gather(
                                G[:, s * (SUBJ // 128) : (s + 1) * (SUBJ // 128), :],
                                xp_ext[:],
                                it[:, s * (SUBJ // 16) : (s + 1) * (SUBJ // 16)],
                                SUBJ,
                                SUBJ,
                                256,
                                queue_num=blk % NQ,
                            )

                        # t[p, (g,tb), c, lr] = G * w[(g,tb), lr] (c broadcast)
                        g4 = G[:].rearrange("p g i -> p (g i)").rearrange(
                            "p (a c lr) -> p a c lr", c=C, lr=2
                        )
                        w3 = wt.rearrange("p (a lr) -> p a lr", lr=2)
                        w5 = w3.unsqueeze(2).broadcast_to((128, 2 * GC, C, 2))
                        nc.vector.tensor_mul(g4, g4, w5)

                        # fold top+bottom rows (contiguous 128-elem halves)
                        u = up.tile([128, GC, 128], DT.float16, tag="u")
                        # add as InstTensorScalarPtr (4x-capable perf class)
                        nc.vector.scalar_tensor_tensor(
                            u[:],
                            G[:, :, 0:128],
                            1.0,
                            G[:, :, 128:256],
                            mybir.AluOpType.mult,
                            mybir.AluOpType.add,
                        )

                        # fold left+right (strided pairs), emit f32, c-major
                        v = vp.tile([128, C, GC], DT.float32, tag="v")
                        u4 = u[:].rearrange("p g (c lr) -> p g c lr", lr=2)
                        vt = v[:].rearrange("p c g -> p g c")
                        nc.vector.tensor_add(vt, u4[:, :, :, 0], u4[:, :, :, 1])

                        nc.scalar.dma_start(out=out_v[t, q], in_=v[:])
    nc.compile()
    _cache[ckey] = nc
    return nc


def _host_prep(x, offset):
    """xpack blocks, gather indices (wrapped+replicated), corner weights."""
    Bn = offset.shape[0]
    off = offset.reshape(Bn, K, 2, H, W)
    ky = np.repeat(np.arange(3), 3)[None, :, None, None]
    kx = np.tile(np.arange(3), 3)[None, :, None, None]
    hs = np.arange(H)[None, None, :, None]
    ws = np.arange(W)[None, None, None, :]
    py = (ky - 1 + hs) + off[:, :, 0]
    px = (kx - 1 + ws) + off[:, :, 1]
    y0 = np.floor(py)
    x0 = np.floor(px)
    ly = (py - y0).astype(np.float32)
    lx = (px - x0).astype(np.float32)
    hy = 1.0 - ly
    hx = 1.0 - lx
    y0i = y0.astype(np.int64)
    x0i = x0.astype(np.int64)

    wy0 = hy * ((y0i >= 0) & (y0i < H))
    wy1 = ly * ((y0i + 1 >= 0) & (y0i + 1 < H))
    swap = y0i == -1
    wy0 = np.where(swap, wy1, wy0)
    wy1 = np.where(swap, 0.0, wy1)
    yc = np.clip(y0i, 0, H - 1)

    wx0 = hx * ((x0i >= 0) & (x0i < W))
    wx1 = lx * ((x0i + 1 >= 0) & (x0i + 1 < W))
    swap = x0i == -1
    wx0 = np.where(swap, wx1, wx0)
    wx1 = np.where(swap, 0.0, wx1)
    xc = np.clip(x0i, 0, W - 1)

    idx = (yc * W + xc).reshape(Bn, K, HW)

    # w4[..., tb*2+lr]
    w4 = np.stack(
        [wy0 * wx0, wy0 * wx1, wy1 * wx0, wy1 * wx1], axis=-1
    ).reshape(Bn, K, HW, 4)
    # sample j = q*4096 + p*32 + g  ->  w_ext[p, (t,q)*GC*4 + g*4 + qq]
    w6 = w4.reshape(Bn, K, NQ, 128, GC, 4)
    w_dev = np.ascontiguousarray(
        w6.transpose(0, 3, 1, 2, 4, 5).reshape(Bn, 128, NCHUNK * GC * 4)
    ).astype(np.float16)

    # gather list order jj = g*128 + p  ->  slot (p, g)
    idxh = idx.reshape(Bn, K, NQ, 128, GC)
    lst = idxh.transpose(0, 1, 2, 4, 3).reshape(Bn, K, NQ, J)
    # wrap per 16 partitions: list[jj] at partition jj%16, col jj//16
    wrapped = lst.reshape(Bn, K, NQ, J // 16, 16).transpose(0, 1, 2, 4, 3)
    rep = np.broadcast_to(
        wrapped[:, :, :, None, :, :], (Bn, K, NQ, 8, 16, J // 16)
    ).reshape(Bn, K, NQ, 128, J // 16)
    idx_dev = np.ascontiguousarray(
        rep.transpose(0, 3, 1, 2, 4).reshape(Bn, 128, NCHUNK * (J // 16))
    ).astype(np.int16)

    # xpack[b, s, tb*128 + c*2 + lr]
    yi = np.minimum(np.arange(H) + 1, H - 1)
    xi = np.minimum(np.arange(W) + 1, W - 1)
    a00 = x
    a01 = x[:, :, :, xi]
    a10 = x[:, :, yi, :]
    a11 = a10[:, :, :, xi]
    arr = np.stack([a00, a01, a10, a11], axis=2).reshape(Bn, C, 2, 2, HW)
    xp = np.ascontiguousarray(
        arr.transpose(0, 4, 2, 1, 3).reshape(Bn, HW, 256)
    ).astype(np.float16)
    return xp, idx_dev, w_dev


def kernel(x, offset):
    x = np.ascontiguousarray(x, dtype=np.float32)
    offset = np.ascontiguousarray(offset, dtype=np.float32)
    xp, idx_dev, w_dev = _host_prep(x, offset)
    nc = _build_nc()
    in_maps = [
        {"xp": xp[b], "idx": idx_dev[b], "w": w_dev[b]} for b in range(B)
    ]
    res = run_bass_kernel_spmd(nc, in_maps, list(range(B)))
    out = np.stack([res.results[b]["out"] for b in range(B)], axis=0)
    return np.ascontiguousarray(out, dtype=np.float32)


# revision 13
# speedup vs baseline: 1.1588x; 1.1588x over previous
"""DeformUnfold (3x3, pad 1, stride 1, dil 1, DG 1) on TRN2, batch-parallel
over 8 NeuronCores.

Input  x      [8, 64, 128, 128] f32
       offset [8, 18, 128, 128] f32
Output        [8, 576, 16384]   f32  (C*K x Ho*Wo unfold, channel-major)

Per core (= one batch element), sample-major SWDGE gather design:
 - Host packs xpack[s, tb*128 + c*2 + lr] = x[c, min(y+tb,127), min(x+lr,127)]
   (s = y*128+x): the full 2x2 bilinear footprint of spatial position s as
   one contiguous 512B fp16 block in HBM.
 - Host folds offsets into per-sample int16 gather indices (corner block
   row, border-clamped with weight-swap trick) and 4 fp16 corner weights.
 - Device, per quarter-tap chunk of J=4096 samples: 4 dma_gathers of 1024
   indices (SWDGE descriptor-carveout limit) land sample j at SBUF
   partition j%128, slot j//128 -> G[128, 32, 256].  SWDGE gather is
   descriptor-bound (~8ns/desc per queue, ~3.6ns/desc with 4 queues), so
   chunks round-robin over all 4 SWDGE queues with a 4-deep G pool to keep
   every queue busy.  DVE: one in-place tensor_mul against the weight tile
   broadcast across the 64-channel dim via a stride-0 middle AP dim (keeps
   the packed-fp16 2x mode, which only constrains the innermost dim), one
   packed fp16 add folding top+bottom rows, one strided add folding
   left+right that emits f32 directly.
 - Output DMA writes [p][c][g] -> out[c*9+k, q*4096 + p*32 + g].
No SBUF staging of x; gathers read HBM directly.  Per-core: gather 75.5MB
(147456 descriptors), DVE ~295K cycles, out 37.75MB, all overlapped;
descriptor generation on the Pool engine is the bottleneck (~3.5ns/desc
plus ~1us fixed per 1024-desc call; the 1024 limit is the ucode SWDGE
descriptor carveout).  Measured ~0.6-0.8ms/iter vs 2.55ms for the
previous ap_gather design (ap_gather costs ~17ns/index on the Q7 SIMD
read path, ~5x the SWDGE descriptor path).
"""

import contextlib

import numpy as np
import ml_dtypes  # noqa: F401

import concourse.bacc as bacc
import concourse.mybir as mybir
import concourse.tile as tile
from concourse.bass_utils import run_bass_kernel_spmd

B, C, H, W = 8, 64, 128, 128
K = 9
HW = H * W
J = 4096            # gather indices per chunk (quarter tap)
SUBJ = 1024         # indices per dma_gather call (descriptor-carveout limit)
GC = J // 128       # 32 free slots per partition
NQ = 4              # quarter-taps per tap == SWDGE queues
NCHUNK = K * NQ
DT = mybir.dt

_cache = {}


def _build_nc(repeat=1):
    ckey = ("nc", repeat)
    if ckey in _cache:
        return _cache[ckey]
    nc = bacc.Bacc(
        "TRN2", target_bir_lowering=False, debug=False, num_swdge_queues=NQ
    )
    xp_ext = nc.declare_dram_parameter("xp", [HW, 256], DT.float16, isOutput=False)
    idx_ext = nc.declare_dram_parameter(
        "idx", [128, NCHUNK * (J // 16)], DT.int16, isOutput=False
    )
    w_ext = nc.declare_dram_parameter(
        "w", [128, NCHUNK * GC * 4], DT.float16, isOutput=False
    )
    out_ext = nc.declare_dram_parameter("out", [C * K, HW], DT.float32, isOutput=True)
    # out[c*K + k, q*4096 + p*32 + g]
    out_v = out_ext[:].rearrange("(c k) (q p g) -> k q p c g", k=K, q=NQ, p=128)

    with tile.TileContext(nc) as tc:
        with (
            tc.tile_pool(name="gp", bufs=NQ) as gp,
            tc.tile_pool(name="ip", bufs=4) as ip,
            tc.tile_pool(name="wp", bufs=4) as wp,
            tc.tile_pool(name="up", bufs=4) as up,
            tc.tile_pool(name="vp", bufs=4) as vp,
        ):
            loop_cm = (
                tc.For_i(0, repeat, 1) if repeat > 1 else contextlib.nullcontext()
            )
            with loop_cm:
                for t in range(K):
                    for q in range(NQ):
                        blk = t * NQ + q
                        itt = ip.tile([128, J // 16], DT.int16, tag="idx")
                        nc.sync.dma_start(
                            out=itt[:],
                            in_=idx_ext[:, blk * (J // 16) : (blk + 1) * (J // 16)],
                        )
                        it = itt[:]
                        wtt = wp.tile([128, GC * 4], DT.float16, tag="w")
                        nc.sync.dma_start(
                            out=wtt[:],
                            in_=w_ext[:, blk * (GC * 4) : (blk + 1) * (GC * 4)],
                        )
                        wt = wtt[:]
                        G = gp.tile([128, GC, 256], DT.float16, tag="g")
                        for s in range(J // SUBJ):
                            nc.gpsimd.dma_gather(
                                G[:, s * (SUBJ // 128) : (s + 1) * (SUBJ // 128), :],
                                xp_ext[:],
                                it[:, s * (SUBJ // 16) : (s + 1) * (SUBJ // 16)],
                                SUBJ,
                                SUBJ,
                                256,
                                queue_num=blk % NQ,
                            )

                        # t[p, (g,tb), c, lr] = G * w[(g,tb), lr] (c broadcast)
                        g4 = G[:].rearrange("p g i -> p (g i)").rearrange(
                            "p (a c lr) -> p a c lr", c=C, lr=2
                        )
                        w3 = wt.rearrange("p (a lr) -> p a lr", lr=2)
                        w5 = w3.unsqueeze(2).broadcast_to((128, 2 * GC, C, 2))
                        nc.vector.tensor_mul(g4, g4, w5)

                        # fold top+bottom rows (contiguous 128-elem halves)
                        u = up.tile([128, GC, 128], DT.float16, tag="u")
                        # add as InstTensorScalarPtr (4x-capable perf class)
                        nc.vector.scalar_tensor_tensor(
                            u[:],
                            G[:, :, 0:128],
                            1.0,
                            G[:, :, 128:256],
                            mybir.AluOpType.mult,
                            mybir.AluOpType.add,
                        )

                        # fold left+right (strided pairs), emit f32, c-major
                        v = vp.tile([128, C, GC], DT.float32, tag="v")
                        u4 = u[:].rearrange("p g (c lr) -> p g c lr", lr=2)
                        vt = v[:].rearrange("p c g -> p g c")
                        nc.vector.tensor_add(vt, u4[:, :, :, 0], u4[:, :, :, 1])

                        nc.scalar.dma_start(out=out_v[t, q], in_=v[:])
    nc.compile()
    _cache[ckey] = nc
    return nc


def _host_prep(x, offset):
    """xpack blocks, gather indices (wrapped+replicated), corner weights."""
    Bn = offset.shape[0]
    off = offset.reshape(Bn, K, 2, H, W)
    ky = np.repeat(np.arange(3), 3)[None, :, None, None]
    kx = np.tile(np.arange(3), 3)[None, :, None, None]
    hs = np.arange(H)[None, None, :, None]
    ws = np.arange(W)[None, None, None, :]
    py = (ky - 1 + hs) + off[:, :, 0]
    px = (kx - 1 + ws) + off[:, :, 1]
    y0 = np.floor(py)
    x0 = np.floor(px)
    ly = (py - y0).astype(np.float32)
    lx = (px - x0).astype(np.float32)
    hy = 1.0 - ly
    hx = 1.0 - lx
    y0i = y0.astype(np.int64)
    x0i = x0.astype(np.int64)

    wy0 = hy * ((y0i >= 0) & (y0i < H))
    wy1 = ly * ((y0i + 1 >= 0) & (y0i + 1 < H))
    swap = y0i == -1
    wy0 = np.where(swap, wy1, wy0)
    wy1 = np.where(swap, 0.0, wy1)
    yc = np.clip(y0i, 0, H - 1)

    wx0 = hx * ((x0i >= 0) & (x0i < W))
    wx1 = lx * ((x0i + 1 >= 0) & (x0i + 1 < W))
    swap = x0i == -1
    wx0 = np.where(swap, wx1, wx0)
    wx1 = np.where(swap, 0.0, wx1)
    xc = np.clip(x0i, 0, W - 1)

    idx = (yc * W + xc).reshape(Bn, K, HW)

    # w4[..., tb*2+lr]
    w4 = np.stack(
        [wy0 * wx0, wy0 * wx1, wy1 * wx0, wy1 * wx1], axis=-1
    ).reshape(Bn, K, HW, 4)
    # sample j = q*4096 + p*32 + g  ->  w_ext[p, (t,q)*GC*4 + g*4 + qq]
    w6 = w4.reshape(Bn, K, NQ, 128, GC, 4)
    w_dev = np.ascontiguousarray(
        w6.transpose(0, 3, 1, 2, 4, 5).reshape(Bn, 128, NCHUNK * GC * 4)
    ).astype(np.float16)

    # gather list order jj = g*128 + p  ->  slot (p, g)
    idxh = idx.reshape(Bn, K, NQ, 128, GC)
    lst = idxh.transpose(0, 1, 2, 4, 3).reshape(Bn, K, NQ, J)
    # wrap per 16 partitions: list[jj] at partition jj%16, col jj//16
    wrapped = lst.reshape(Bn, K, NQ, J // 16, 16).transpose(0, 1, 2, 4, 3)
    rep = np.broadcast_to(
        wrapped[:, :, :, None, :, :], (Bn, K, NQ, 8, 16, J // 16)
    ).reshape(Bn, K, NQ, 128, J // 16)
    idx_dev = np.ascontiguousarray(
        rep.transpose(0, 3, 1, 2, 4).reshape(Bn, 128, NCHUNK * (J // 16))
    ).astype(np.int16)

    # xpack[b, s, tb*128 + c*2 + lr]
    yi = np.minimum(np.arange(H) + 1, H - 1)
    xi = np.minimum(np.arange(W) + 1, W - 1)
    a00 = x
    a01 = x[:, :, :, xi]
    a10 = x[:, :, yi, :]
    a11 = a10[:, :, :, xi]
    arr = np.stack([a00, a01, a10, a11], axis=2).reshape(Bn, C, 2, 2, HW)
    xp = np.ascontiguousarray(
        arr.transpose(0, 4, 2, 1, 3).reshape(Bn, HW, 256)
    ).astype(np.float16)
    return xp, idx_dev, w_dev


def kernel(x, offset):
    x = np.ascontiguousarray(x, dtype=np.float32)
    offset = np.ascontiguousarray(offset, dtype=np.float32)
    xp, idx_dev, w_dev = _host_prep(x, offset)
    nc = _build_nc()
    in_maps = [
        {"xp": xp[b], "idx": idx_dev[b], "w": w_dev[b]} for b in range(B)
    ]
    res = run_bass_kernel_spmd(nc, in_maps, list(range(B)))
    out = np.stack([res.results[b]["out"] for b in range(B)], axis=0)
    return np.ascontiguousarray(out, dtype=np.float32)
